# revision 1
# baseline (speedup 1.0000x reference)
"""GraphSAGE (2x SAGEConv + global mean pool + FC + sigmoid) on 8 TRN2 NeuronCores.

Strategy
--------
The SAGEConv projection commutes with mean aggregation:
    h = relu([x | mean_nbr(x)] @ W1) = relu(x @ W1_top + mean_nbr(x @ W1_bot))
so we project to DIM=10 first and only ever gather 10(->16 padded)-float rows
instead of 128-float rows.

Sharding: nodes are globally sorted by in-degree (desc) and dealt round-robin
to the 8 cores, so every core has an identical per-block degree profile ->
one SPMD program with compile-time-uniform gather counts per 128-node block.
Each core aggregates its own 12544 nodes (padded-CSR slot tables built on
host); the projected tables (y1, y2, h2) are exchanged with AllGather
collectives. Pooling: batch is sorted, so each core pools 125 whole graphs
from the AllGather'd h2 table and emits its [125,1] output shard.
"""

import numpy as np

N = 100_000
B = 1000
F_IN = 128
DIM = 10
NCORES = 8
PERC = 12544            # nodes per core (98 blocks of 128); 12500 real + 44 dummy
NB = PERC // 128        # 98
NTOT = PERC * NCORES    # 100352
ZR = NTOT               # zero-row index in the gather tables
TABR = NTOT + 1         # table rows incl. zero row
F16 = 16                # padded feature width

_CACHE: dict = {}


# ----------------------------------------------------------------- host prep
def _host_prep(edge_index, batch):
    src = np.asarray(edge_index[0], dtype=np.int64)
    dst = np.asarray(edge_index[1], dtype=np.int64)
    batch = np.asarray(batch, dtype=np.int64)

    deg = np.bincount(dst, minlength=N).astype(np.int64)          # in-degree
    deg_ext = np.concatenate([deg, np.full(NTOT - N, -1, np.int64)])
    order = np.argsort(-deg_ext, kind="stable")                   # rank -> orig
    rank = np.empty(NTOT, np.int64)
    rank[order] = np.arange(NTOT)
    core_of = rank % NCORES
    local_of = rank // NCORES
    pid = core_of * PERC + local_of                               # orig -> table row

    # per-block gather counts (identical across cores by construction)
    Ks = np.zeros(NB, np.int64)
    d_sorted = np.maximum(deg_ext[order], 0)                      # by rank
    blk_of_rank = (np.arange(NTOT) // NCORES) // 128
    np.maximum.at(Ks, blk_of_rank, d_sorted)
    cumK = np.concatenate([[0], np.cumsum(Ks)]).astype(np.int64)
    TOTK = int(cumK[-1])

    # slot tables: sidx[c][part, cumK[b]+k] = pid[src of k-th edge into node]
    dpid = pid[dst]
    eo = np.argsort(dpid, kind="stable")
    sd = dpid[eo]
    sp = pid[src[eo]].astype(np.int32)
    node_start = np.searchsorted(sd, np.arange(NTOT))
    k_within = np.arange(len(sd)) - node_start[sd]
    c_e = sd // PERC
    l_e = sd % PERC
    col_e = cumK[l_e // 128] + k_within
    sidx = np.full((NCORES, 128, TOTK), ZR, np.int32)
    sidx[c_e, l_e % 128, col_e] = sp

    # recip degrees [128, NB] per core (partition-major for one straight DMA)
    recips = np.zeros((NCORES, 128, NB), np.float32)
    rd = 1.0 / np.maximum(deg_ext, 1).astype(np.float32)
    for c in range(NCORES):
        recips[c] = rd[order[c::NCORES]].reshape(NB, 128).T

    # pooling tables
    cnt = np.bincount(batch, minlength=B).astype(np.int64)
    starts = np.concatenate([[0], np.cumsum(cnt)])
    KP = int(cnt.max())
    GPC = B // NCORES  # 125 graphs per core
    pool_sidx = np.full((NCORES, 128, KP), ZR, np.int32)
    pool_recip = np.zeros((NCORES, 128), np.float32)
    for g in range(B):
        c, p = g // GPC, g % GPC
        nodes = np.arange(starts[g], starts[g + 1])
        pool_sidx[c, p, : len(nodes)] = pid[nodes].astype(np.int32)
        pool_recip[c, p] = 1.0 / max(len(nodes), 1)

    return dict(
        order=order, Ks=[int(v) for v in Ks], cumK=cumK, TOTK=TOTK, KP=KP,
        sidx=sidx, recips=recips, pool_sidx=pool_sidx, pool_recip=pool_recip,
    )


def _host_inputs(prep, x, W1, W2, Wfc):
    x = np.asarray(x, np.float32)
    W1 = np.asarray(W1, np.float32)
    W2 = np.asarray(W2, np.float32)
    Wfc = np.asarray(Wfc, np.float32)
    x_ext = np.concatenate([x, np.zeros((NTOT - N, F_IN), np.float32)], 0)
    W1cat = np.concatenate([W1[:F_IN], W1[F_IN:]], axis=1)        # [128, 20]
    W2cat = np.zeros((F16, 2 * DIM), np.float32)
    W2cat[:DIM, :DIM] = W2[:DIM]
    W2cat[:DIM, DIM:] = W2[DIM:]
    wfc_t = np.zeros((128, F16), np.float32)
    wfc_t[:, :DIM] = Wfc[:, 0]

    in_maps = []
    order = prep["order"]
    for c in range(NCORES):
        oc = order[c::NCORES]
        in_maps.append({
            "xT": np.ascontiguousarray(x_ext[oc].T),              # [128, 12544]
            "sidx": np.ascontiguousarray(prep["sidx"][c]),        # [128, TOTK]
            "recips": np.ascontiguousarray(prep["recips"][c]),    # [128, NB]
            "W1cat": W1cat,
            "W2cat": W2cat,
            "wfc": wfc_t,
            "psidx": np.ascontiguousarray(prep["pool_sidx"][c]),  # [128, KP]
            "precip": prep["pool_recip"][c].reshape(128, 1).copy(),
        })
    return in_maps


# -------------------------------------------------------------- kernel build
def _build_bass(Ks, TOTK, KP):
    import concourse.bass as bass
    import concourse.mybir as mybir
    import concourse.tile as tile
    from concourse import bacc
    from concourse.masks import make_identity

    f32 = mybir.dt.float32
    i32 = mybir.dt.int32
    AF = mybir.ActivationFunctionType
    ALU = mybir.AluOpType
    AX = mybir.AxisListType
    RG = [list(range(NCORES))]
    cumK = np.concatenate([[0], np.cumsum(Ks)]).astype(np.int64)

    nc = bacc.Bacc(num_devices=NCORES)

    xT = nc.dram_tensor("xT", [128, PERC], f32, kind="ExternalInput")
    sidx = nc.dram_tensor("sidx", [128, TOTK], i32, kind="ExternalInput")
    recips = nc.dram_tensor("recips", [128, NB], f32, kind="ExternalInput")
    W1cat = nc.dram_tensor("W1cat", [128, 2 * DIM], f32, kind="ExternalInput")
    W2cat = nc.dram_tensor("W2cat", [F16, 2 * DIM], f32, kind="ExternalInput")
    wfc = nc.dram_tensor("wfc", [128, F16], f32, kind="ExternalInput")
    psidx = nc.dram_tensor("psidx", [128, KP], i32, kind="ExternalInput")
    precip = nc.dram_tensor("precip", [128, 1], f32, kind="ExternalInput")
    out = nc.dram_tensor("out", [128, 1], f32, kind="ExternalOutput")

    ag_in = [nc.dram_tensor(f"ag{i}_in", [PERC, F16], f32, kind="Internal")
             for i in range(3)]
    ag_out = [nc.dram_tensor(f"ag{i}_out", [TABR, F16], f32, kind="Internal",
                             addr_space="Shared") for i in range(3)]

    with tile.TileContext(nc) as tc:
        with (
            tc.tile_pool(name="const", bufs=1) as cpool,
            tc.tile_pool(name="store", bufs=1) as spool,
            tc.tile_pool(name="work", bufs=3) as wpool,
            tc.tile_pool(name="msg", bufs=2) as mpool,
            tc.tile_pool(name="psum", bufs=4, space="PSUM") as ppool,
        ):
            # ---- constants / persistent inputs
            ident = cpool.tile([128, 128], f32)
            make_identity(nc, ident[:])
            w1_sb = cpool.tile([128, 2 * DIM], f32)
            nc.sync.dma_start(out=w1_sb[:], in_=W1cat[:, :])
            w2_sb = cpool.tile([F16, 2 * DIM], f32)
            nc.sync.dma_start(out=w2_sb[:], in_=W2cat[:, :])
            wfc_sb = cpool.tile([128, F16], f32)
            nc.sync.dma_start(out=wfc_sb[:], in_=wfc[:, :])
            prc_sb = cpool.tile([128, 1], f32)
            nc.sync.dma_start(out=prc_sb[:], in_=precip[:, :])
            xT_sb = cpool.tile([128, PERC], f32)
            nc.sync.dma_start(out=xT_sb[:], in_=xT[:, :])
            sidx_sb = cpool.tile([128, TOTK], i32)
            nc.sync.dma_start(out=sidx_sb[:], in_=sidx[:, :])
            rcp_sb = cpool.tile([128, NB], f32)
            nc.sync.dma_start(out=rcp_sb[:], in_=recips[:, :])
            zero16 = cpool.tile([1, F16], f32)
            nc.vector.memset(zero16[:], 0.0)
            # zero rows of all three tables
            for t in range(3):
                nc.sync.dma_start(out=ag_out[t][NTOT:TABR, :], in_=zero16[:])

            # ---- persistent stores
            s1_all = spool.tile([128, NB * DIM], f32)    # x @ W1_top
            h_all = spool.tile([128, NB * F16], f32)     # relu layer-1 out (padded)
            z_all = spool.tile([128, NB * DIM], f32)     # h @ W2_top
            nc.vector.memset(h_all[:], 0.0)

            # ================= phase A: layer-1 projection =================
            y1_all = spool.tile([128, NB * F16], f32)
            nc.vector.memset(y1_all[:], 0.0)
            for b in range(NB):
                ps = ppool.tile([128, 2 * DIM], f32, tag="proj")
                nc.tensor.matmul(out=ps[:], lhsT=xT_sb[:, 128 * b:128 * (b + 1)],
                                 rhs=w1_sb[:], start=True, stop=True)
                nc.scalar.activation(out=s1_all[:, DIM * b:DIM * (b + 1)],
                                     in_=ps[:, :DIM], func=AF.Copy)
                nc.vector.tensor_copy(out=y1_all[:, F16 * b:F16 * b + DIM],
                                      in_=ps[:, DIM:])
            # one big strided DMA: SBUF [128, NB*16] -> DRAM rows (128b+p)
            nc.sync.dma_start(
                out=ag_in[0][:, :].rearrange("(b p) f -> p b f", p=128),
                in_=y1_all[:].rearrange("p (b f) -> p b f", f=F16))

            nc.gpsimd.collective_compute(
                "AllGather", mybir.AluOpType.bypass, replica_groups=RG,
                ins=[ag_in[0][:, :]], outs=[ag_out[0][0:NTOT, :]])

            # ================= phase B/D: aggregation ======================
            def aggregate(table, src_store, src_w, dst_store, relu):
                """dst = (relu?)(src_store[b] + mean_aggr @ ...) per block."""
                for b in range(NB):
                    K = Ks[b]
                    base = int(cumK[b])
                    if K > 0:
                        msg = mpool.tile([128, K * F16], f32, tag="msg")
                        for k in range(K):
                            nc.gpsimd.indirect_dma_start(
                                out=msg[:, F16 * k:F16 * (k + 1)],
                                out_offset=None,
                                in_=table[:, :],
                                in_offset=bass.IndirectOffsetOnAxis(
                                    ap=sidx_sb[:, base + k:base + k + 1], axis=0),
                            )
                        agg = wpool.tile([128, F16], f32, tag="agg")
                        nc.vector.tensor_reduce(
                            out=agg[:],
                            in_=msg[:].rearrange("p (k f) -> p f k", k=K, f=F16),
                            axis=AX.X, op=ALU.add)
                        # mean + add self-projection
                        nc.vector.tensor_scalar_mul(
                            agg[:, :DIM], agg[:, :DIM], rcp_sb[:, b:b + 1])
                        nc.vector.tensor_add(
                            out=agg[:, :DIM],
                            in0=agg[:, :DIM],
                            in1=src_store[:, src_w * b:src_w * b + DIM])
                        src_ap = agg[:, :DIM]
                    else:
                        src_ap = src_store[:, src_w * b:src_w * b + DIM]
                    nc.scalar.activation(
                        out=dst_store[:, F16 * b:F16 * b + DIM], in_=src_ap,
                        func=AF.Relu if relu else AF.Copy)

            aggregate(ag_out[0], s1_all, DIM, h_all, relu=True)

            # ================= phase C: layer-2 projection =================
            y2_all = spool.tile([128, NB * F16], f32)
            nc.vector.memset(y2_all[:], 0.0)
            for b in range(NB):
                psT = ppool.tile([F16, 128], f32, tag="psT")
                nc.tensor.transpose(out=psT[:], in_=h_all[:, F16 * b:F16 * (b + 1)],
                                    identity=ident[:])
                hT = wpool.tile([F16, 128], f32, tag="hT")
                nc.vector.tensor_copy(out=hT[:], in_=psT[:])
                ps2 = ppool.tile([128, 2 * DIM], f32, tag="proj")
                nc.tensor.matmul(out=ps2[:], lhsT=hT[:], rhs=w2_sb[:],
                                 start=True, stop=True)
                nc.scalar.activation(out=z_all[:, DIM * b:DIM * (b + 1)],
                                     in_=ps2[:, :DIM], func=AF.Copy)
                nc.vector.tensor_copy(out=y2_all[:, F16 * b:F16 * b + DIM],
                                      in_=ps2[:, DIM:])
            nc.sync.dma_start(
                out=ag_in[1][:, :].rearrange("(b p) f -> p b f", p=128),
                in_=y2_all[:].rearrange("p (b f) -> p b f", f=F16))

            nc.gpsimd.collective_compute(
                "AllGather", mybir.AluOpType.bypass, replica_groups=RG,
                ins=[ag_in[1][:, :]], outs=[ag_out[1][0:NTOT, :]])

            # ---- layer-2 aggregation -> h2 into h_all (reuse), then AG
            h2_all = spool.tile([128, NB * F16], f32)
            nc.vector.memset(h2_all[:], 0.0)
            aggregate(ag_out[1], z_all, DIM, h2_all, relu=False)
            nc.sync.dma_start(
                out=ag_in[2][:, :].rearrange("(b p) f -> p b f", p=128),
                in_=h2_all[:].rearrange("p (b f) -> p b f", f=F16))
            nc.gpsimd.collective_compute(
                "AllGather", mybir.AluOpType.bypass, replica_groups=RG,
                ins=[ag_in[2][:, :]], outs=[ag_out[2][0:NTOT, :]])

            # ================= phase E: pooling + FC + sigmoid =============
            pix = spool.tile([128, KP], i32)
            nc.sync.dma_start(out=pix[:], in_=psidx[:, :])
            pmsg = spool.tile([128, KP * F16], f32)
            for k in range(KP):
                nc.gpsimd.indirect_dma_start(
                    out=pmsg[:, F16 * k:F16 * (k + 1)],
                    out_offset=None,
                    in_=ag_out[2][:, :],
                    in_offset=bass.IndirectOffsetOnAxis(ap=pix[:, k:k + 1], axis=0),
                )
            pool = spool.tile([128, F16], f32)
            nc.vector.tensor_reduce(
                out=pool[:],
                in_=pmsg[:].rearrange("p (k f) -> p f k", k=KP, f=F16),
                axis=AX.X, op=ALU.add)
            nc.vector.tensor_scalar_mul(pool[:], pool[:], prc_sb[:])
            nc.vector.tensor_mul(out=pool[:], in0=pool[:], in1=wfc_sb[:])
            logit = spool.tile([128, 1], f32)
            nc.vector.tensor_reduce(out=logit[:], in_=pool[:], axis=AX.X, op=ALU.add)
            res = spool.tile([128, 1], f32)
            nc.scalar.activation(out=res[:], in_=logit[:], func=AF.Sigmoid)
            nc.sync.dma_start(out=out[:, :], in_=res[:])

    nc.finalize()
    return nc


# ------------------------------------------------------------------- driver
def kernel(**inputs) -> np.ndarray:
    from concourse.bass_utils import run_bass_kernel_spmd

    edge_index = np.asarray(inputs["edge_index"])
    batch = np.asarray(inputs["batch"])
    key = (edge_index.shape, int(edge_index[:, ::997].sum()), int(batch[::997].sum()))
    if key not in _CACHE:
        prep = _host_prep(edge_index, batch)
        nc = _build_bass(prep["Ks"], prep["TOTK"], prep["KP"])
        _CACHE[key] = (prep, nc)
    prep, nc = _CACHE[key]

    in_maps = _host_inputs(prep, inputs["x"], inputs["W1"], inputs["W2"],
                           inputs["Wfc"])
    res = run_bass_kernel_spmd(nc, in_maps, core_ids=list(range(NCORES)))
    gpc = B // NCORES
    parts = [res.results[c]["out"][:gpc, :] for c in range(NCORES)]
    return np.concatenate(parts, axis=0).astype(np.float32)

